# revision 3
# baseline (speedup 1.0000x reference)
"""Trainium2 Bass kernel for nn_IrrepsToHessian (gnn_message_passing).

Math: for pair e=(cfg c, atom a, atom b), the reference computes
  irr[e,k]  = x_a . B_k . x_b          (B_k from tp_weights x Wigner-3j)
  cart[e]   = sum_k irr[e,k] * QCART[k]          (3x3)
  out[c,a,b] = 0.5*(cart[c,a,b] + cart[c,b,a]^T)
Folding QCART and the symmetrization into the constants gives
  out[c,a,b,i,j] = x_a . C_ij . x_b,
  C_ij = 0.5 * sum_k (B_k*QC[k,i,j] + B_k^T*QC[k,j,i]),
so per config the whole pair block is 9 small matmuls
  F_ij = X_c @ C_ij @ X_c^T   (64x144 @ 144x144 @ 144x64).
Sharding: 8 configs per core (config axis data-parallel, no comms).
"""
import math
import sys

import numpy as np

sys.path.insert(0, "/opt/trn_rl_repo")

# ---- static problem configuration ----
MUL = 16
B_CFG = 64
N_ATOM = 64
N_NODES = B_CFG * N_ATOM          # 4096
N_PAIRS = B_CFG * N_ATOM * N_ATOM  # 262144
FDIM = MUL * 9                     # 144
N_CORES = 8
CFG_PER_CORE = B_CFG // N_CORES    # 8
ROWS_PER_CORE = CFG_PER_CORE * N_ATOM  # 512

PATHS = [(0, 0, 0), (1, 1, 0), (2, 2, 0),
         (0, 1, 1), (1, 0, 1), (1, 1, 1), (1, 2, 1), (2, 1, 1), (2, 2, 1),
         (0, 2, 2), (2, 0, 2), (1, 1, 2), (1, 2, 2), (2, 1, 2), (2, 2, 2)]


# ---- Wigner 3j / QCART constants (identical math to the reference) ----
def _su2_cg(j1, m1, j2, m2, j3, m3):
    if m3 != m1 + m2:
        return 0.0
    f = math.factorial
    vmin = max(-j1 + j2 + m3, -j1 + m1, 0)
    vmax = min(j2 + j3 + m1, j3 - j1 + j2, j3 + m3)
    C = math.sqrt((2 * j3 + 1) * f(j3 + j1 - j2) * f(j3 - j1 + j2) * f(j1 + j2 - j3)
                  * f(j3 + m3) * f(j3 - m3)
                  / (f(j1 + j2 + j3 + 1) * f(j1 - m1) * f(j1 + m1) * f(j2 - m2) * f(j2 + m2)))
    S = 0.0
    for v in range(vmin, vmax + 1):
        S += (-1) ** (v + j2 + m2) * f(j2 + j3 + m1 - v) * f(j1 - m1 + v) / (
            f(v) * f(j3 - j1 + j2 - v) * f(j3 + m3 - v) * f(v + j1 - j2 - m3))
    return C * S


def _q_real_to_complex(l):
    q = np.zeros((2 * l + 1, 2 * l + 1), dtype=np.complex128)
    for m in range(-l, 0):
        q[l + m, l + abs(m)] = 1 / 2 ** 0.5
        q[l + m, l - abs(m)] = -1j / 2 ** 0.5
    q[l, l] = 1.0
    for m in range(1, l + 1):
        q[l + m, l + abs(m)] = (-1) ** m / 2 ** 0.5
        q[l + m, l - abs(m)] = 1j * (-1) ** m / 2 ** 0.5
    return (-1j) ** l * q


def _wigner_3j(l1, l2, l3):
    C = np.zeros((2 * l1 + 1, 2 * l2 + 1, 2 * l3 + 1))
    for m1 in range(-l1, l1 + 1):
        for m2 in range(-l2, l2 + 1):
            m3 = m1 + m2
            if abs(m3) <= l3:
                C[l1 + m1, l2 + m2, l3 + m3] = _su2_cg(l1, m1, l2, m2, l3, m3)
    Q1, Q2, Q3 = _q_real_to_complex(l1), _q_real_to_complex(l2), _q_real_to_complex(l3)
    Cr = np.einsum('ij,kl,mn,ikm->jln', Q1, Q2, np.conj(Q3), C.astype(np.complex128))
    assert np.abs(Cr.imag).max() < 1e-6
    Cr = Cr.real
    return Cr / np.linalg.norm(Cr)


W3J = {p: _wigner_3j(*p) for p in set(PATHS)}
FAN = {lo: sum(1 for p in PATHS if p[2] == lo) * MUL * MUL for lo in (0, 1, 2)}
ALPHA = {lo: math.sqrt(2 * lo + 1) / math.sqrt(FAN[lo]) for lo in (0, 1, 2)}
QCART = np.concatenate([math.sqrt(2 * l + 1) * np.transpose(_wigner_3j(1, 1, l), (2, 0, 1))
                        for l in (0, 1, 2)], axis=0)  # (9, 3, 3)

KOFF = {0: 0, 1: 1, 2: 4}
FOFF = {0: 0, 1: MUL, 2: 4 * MUL}
IDIM = {0: 1, 1: 3, 2: 5}


def _build_B(tp_weights):
    """B[k, fa, fb]: irr[e,k] = x_a . B_k . x_b  (k = 9 irrep components)."""
    B = np.zeros((9, FDIM, FDIM), np.float64)
    for p, (l1, l2, lo) in enumerate(PATHS):
        w3j = W3J[(l1, l2, lo)]                      # (2l1+1, 2l2+1, 2lo+1)
        Wp = tp_weights[p].astype(np.float64)        # (u, v)
        contrib = np.einsum('uv,ijk->kuivj', Wp, w3j) * ALPHA[lo]
        B[KOFF[lo]:KOFF[lo] + IDIM[lo],
          FOFF[l1]:FOFF[l1] + MUL * IDIM[l1],
          FOFF[l2]:FOFF[l2] + MUL * IDIM[l2]] += contrib.reshape(
              IDIM[lo], MUL * IDIM[l1], MUL * IDIM[l2])
    return B


def _build_CT(tp_weights):
    """CT[(i*3+j), fb, fa] = C_ij^T with symmetrization folded in."""
    B = _build_B(tp_weights)
    Cfull = np.einsum('kab,kij->ijab', B, QCART)          # (3,3,fa,fb)
    Csym = 0.5 * (Cfull + np.transpose(Cfull, (1, 0, 3, 2)))
    CT = np.transpose(Csym, (0, 1, 3, 2)).reshape(9, FDIM, FDIM)
    return np.ascontiguousarray(CT.astype(np.float32))


def _canonical_layout():
    ii, jj = np.meshgrid(np.arange(N_ATOM, dtype=np.int32),
                         np.arange(N_ATOM, dtype=np.int32), indexing='ij')
    base = (np.arange(B_CFG, dtype=np.int32) * N_ATOM)[:, None, None]
    rows = (base + ii[None]).reshape(-1)
    cols = (base + jj[None]).reshape(-1)
    return np.stack([rows, cols], axis=1)


def _fallback_numpy(node_feats, layout, tp_weights):
    """General-layout path (host): exact reference semantics in numpy."""
    B = _build_B(tp_weights).astype(np.float32)
    x1 = node_feats[layout[:, 0]]
    x2 = node_feats[layout[:, 1]]
    irr = np.einsum('ef,kfg,eg->ek', x1, B, x2, optimize=True)
    cart = np.einsum('ek,kij->eij', irr, QCART.astype(np.float32))
    c = cart.reshape(B_CFG, N_ATOM, N_ATOM, 3, 3)
    c = 0.5 * (c + np.transpose(c, (0, 2, 1, 4, 3)))
    return np.ascontiguousarray(c.reshape(N_PAIRS, 3, 3))


# ---- device kernel ----
_NC = None
LAST_EXEC_NS = None
PROFILE = False


def _install_profile_hook():
    """Self-install the axon NTFF profile hook (test-harness only).

    The agent image's antenv lacks axon_hooks; provide an in-process
    registry module and wire it to the injected libaxon_pjrt.so.
    """
    import types

    if "antenv.axon_hooks" not in sys.modules:
        import antenv
        mod = types.ModuleType("antenv.axon_hooks")
        state = {"h": None}
        mod.set_axon_ntff_profile_hook = lambda h: state.__setitem__("h", h)
        mod.get_axon_ntff_profile_hook = lambda: state["h"]
        sys.modules["antenv.axon_hooks"] = mod
        antenv.axon_hooks = mod
    import antenv.axon_hooks as ah
    if ah.get_axon_ntff_profile_hook() is None:
        from trn_agent_boot.trn_boot import _ntff_profile_via_ctypes
        ah.set_axon_ntff_profile_hook(
            _ntff_profile_via_ctypes("/opt/axon/libaxon_pjrt.so"))
    import concourse.bass_utils as bu
    bu.upload_artifacts = lambda d: d


def _build_nc():
    from concourse import bacc, mybir
    import concourse.tile as tile

    f32 = mybir.dt.float32
    f32r = mybir.dt.float32r

    nc = bacc.Bacc("TRN2", target_bir_lowering=False, debug=False,
                   num_devices=N_CORES)
    xt_ap = nc.dram_tensor("xt", [FDIM, ROWS_PER_CORE], f32r,
                           kind="ExternalInput").ap()
    ct_ap = nc.dram_tensor("ct", [9, FDIM, FDIM], f32r,
                           kind="ExternalInput").ap()
    out_ap = nc.dram_tensor("out", [ROWS_PER_CORE, N_ATOM * 9], f32,
                            kind="ExternalOutput").ap()

    C = CFG_PER_CORE   # 8 configs
    NB = N_ATOM        # 64
    NCOL = NB * 9      # 576 output cols per config

    with tile.TileContext(nc) as tc:
        with tc.tile_pool(name="const", bufs=1) as const_pool, \
             tc.tile_pool(name="u", bufs=1) as u_pool, \
             tc.tile_pool(name="outp", bufs=2) as outp, \
             tc.tile_pool(name="ps1", bufs=2, space="PSUM") as ps1, \
             tc.tile_pool(name="ps2", bufs=2, space="PSUM") as ps2:

            xt0 = const_pool.tile([128, ROWS_PER_CORE], f32r)
            xt1 = const_pool.tile([16, ROWS_PER_CORE], f32r)
            nc.sync.dma_start(xt0[:], xt_ap[0:128, :])
            nc.sync.dma_start(xt1[:], xt_ap[128:FDIM, :])

            ct0 = const_pool.tile([128, 9 * FDIM], f32r)
            ct1 = const_pool.tile([16, 9 * FDIM], f32r)
            for ij in range(9):
                nc.sync.dma_start(ct0[:, ij * FDIM:(ij + 1) * FDIM],
                                  ct_ap[ij, 0:128, :])
                nc.sync.dma_start(ct1[:, ij * FDIM:(ij + 1) * FDIM],
                                  ct_ap[ij, 128:FDIM, :])

            # stage 1: U_ij = C_ij @ X^T for all 8 configs at once
            # u layout: partitions = fa, free = (cfg, b, ij)
            u0 = u_pool.tile([128, C * NB * 9], f32r)
            u1 = u_pool.tile([16, C * NB * 9], f32r)
            u0v = u0[:].rearrange("p (c b j) -> p c b j", c=C, b=NB)
            u1v = u1[:].rearrange("p (c b j) -> p c b j", c=C, b=NB)
            for ij in range(9):
                o = ij * FDIM
                pa = ps1.tile([128, ROWS_PER_CORE], f32, tag="pa")
                nc.tensor.matmul(pa[:], ct0[:, o:o + 128], xt0[:],
                                 start=True, stop=False)
                nc.tensor.matmul(pa[:], ct1[:, o:o + 128], xt1[:],
                                 start=False, stop=True)
                pb = ps1.tile([16, ROWS_PER_CORE], f32, tag="pb")
                nc.tensor.matmul(pb[:], ct0[:, o + 128:o + FDIM], xt0[:],
                                 start=True, stop=False)
                nc.tensor.matmul(pb[:], ct1[:, o + 128:o + FDIM], xt1[:],
                                 start=False, stop=True)
                eng = nc.scalar if ij % 2 == 0 else nc.vector
                eng2 = nc.vector if ij % 2 == 0 else nc.scalar
                pav = pa[:].rearrange("p (c b) -> p c b", c=C)
                pbv = pb[:].rearrange("p (c b) -> p c b", c=C)
                if ij % 2 == 0:
                    eng.copy(u0v[:, :, :, ij], pav)
                    eng2.tensor_copy(u1v[:, :, :, ij], pbv)
                else:
                    eng.tensor_copy(u0v[:, :, :, ij], pav)
                    eng2.copy(u1v[:, :, :, ij], pbv)

            # stage 2: F = X_c @ U_c -> out rows (c,a), cols (b,ij)
            H = NCOL // 2  # 288
            for c in range(C):
                lhs0 = xt0[:, c * NB:(c + 1) * NB]
                lhs1 = xt1[:, c * NB:(c + 1) * NB]
                base = c * NCOL
                pf0 = ps2.tile([NB, H], f32, tag="pf0")
                nc.tensor.matmul(pf0[:], lhs0, u0[:, base:base + H],
                                 start=True, stop=False)
                nc.tensor.matmul(pf0[:], lhs1, u1[:, base:base + H],
                                 start=False, stop=True)
                pf1 = ps2.tile([NB, H], f32, tag="pf1")
                nc.tensor.matmul(pf1[:], lhs0, u0[:, base + H:base + NCOL],
                                 start=True, stop=False)
                nc.tensor.matmul(pf1[:], lhs1, u1[:, base + H:base + NCOL],
                                 start=False, stop=True)
                fo = outp.tile([NB, NCOL], f32)
                if c % 2 == 0:
                    nc.scalar.copy(fo[:, 0:H], pf0[:])
                    nc.vector.tensor_copy(fo[:, H:NCOL], pf1[:])
                else:
                    nc.vector.tensor_copy(fo[:, 0:H], pf0[:])
                    nc.scalar.copy(fo[:, H:NCOL], pf1[:])
                nc.sync.dma_start(out_ap[c * NB:(c + 1) * NB, :], fo[:])

    nc.compile()
    return nc


def kernel(node_feats, layout, tp_weights):
    global _NC, LAST_EXEC_NS
    node_feats = np.ascontiguousarray(np.asarray(node_feats, dtype=np.float32))
    layout = np.asarray(layout, dtype=np.int32)
    tp_weights = np.asarray(tp_weights, dtype=np.float32)

    if not np.array_equal(layout, _canonical_layout()):
        return _fallback_numpy(node_feats, layout, tp_weights)

    CT = _build_CT(tp_weights)

    if _NC is None:
        _NC = _build_nc()

    from concourse.bass_utils import run_bass_kernel_spmd

    if PROFILE:
        try:
            _install_profile_hook()
        except Exception:
            pass

    in_maps = []
    for m in range(N_CORES):
        sl = node_feats[m * ROWS_PER_CORE:(m + 1) * ROWS_PER_CORE, :]
        xt = np.ascontiguousarray(sl.T)          # (144, 512)
        in_maps.append({"xt": xt, "ct": CT})

    res = run_bass_kernel_spmd(_NC, in_maps, list(range(N_CORES)),
                               trace=PROFILE)
    LAST_EXEC_NS = res.exec_time_ns

    out = np.empty((N_PAIRS, 3, 3), dtype=np.float32)
    for m in range(N_CORES):
        blk = res.results[m]["out"]              # (512, 576)
        out[m * ROWS_PER_CORE * N_ATOM:(m + 1) * ROWS_PER_CORE * N_ATOM] = \
            blk.reshape(ROWS_PER_CORE * N_ATOM, 3, 3)
    return out


# revision 4
# speedup vs baseline: 1.0121x; 1.0121x over previous
"""Trainium2 Bass kernel for nn_IrrepsToHessian (gnn_message_passing).

Math: for pair e=(cfg c, atom a, atom b), the reference computes
  irr[e,k]  = x_a . B_k . x_b          (B_k from tp_weights x Wigner-3j)
  cart[e]   = sum_k irr[e,k] * QCART[k]          (3x3)
  out[c,a,b] = 0.5*(cart[c,a,b] + cart[c,b,a]^T)
Folding QCART and the symmetrization into the constants gives
  out[c,a,b,i,j] = x_a . C_ij . x_b,
  C_ij = 0.5 * sum_k (B_k*QC[k,i,j] + B_k^T*QC[k,j,i]),
so per config the whole pair block is 9 small matmuls
  F_ij = X_c @ C_ij @ X_c^T   (64x144 @ 144x144 @ 144x64).
Sharding: 8 configs per core (config axis data-parallel, no comms).
"""
import math
import sys

import numpy as np

sys.path.insert(0, "/opt/trn_rl_repo")

# ---- static problem configuration ----
MUL = 16
B_CFG = 64
N_ATOM = 64
N_NODES = B_CFG * N_ATOM          # 4096
N_PAIRS = B_CFG * N_ATOM * N_ATOM  # 262144
FDIM = MUL * 9                     # 144
N_CORES = 8
CFG_PER_CORE = B_CFG // N_CORES    # 8
ROWS_PER_CORE = CFG_PER_CORE * N_ATOM  # 512

PATHS = [(0, 0, 0), (1, 1, 0), (2, 2, 0),
         (0, 1, 1), (1, 0, 1), (1, 1, 1), (1, 2, 1), (2, 1, 1), (2, 2, 1),
         (0, 2, 2), (2, 0, 2), (1, 1, 2), (1, 2, 2), (2, 1, 2), (2, 2, 2)]


# ---- Wigner 3j / QCART constants (identical math to the reference) ----
def _su2_cg(j1, m1, j2, m2, j3, m3):
    if m3 != m1 + m2:
        return 0.0
    f = math.factorial
    vmin = max(-j1 + j2 + m3, -j1 + m1, 0)
    vmax = min(j2 + j3 + m1, j3 - j1 + j2, j3 + m3)
    C = math.sqrt((2 * j3 + 1) * f(j3 + j1 - j2) * f(j3 - j1 + j2) * f(j1 + j2 - j3)
                  * f(j3 + m3) * f(j3 - m3)
                  / (f(j1 + j2 + j3 + 1) * f(j1 - m1) * f(j1 + m1) * f(j2 - m2) * f(j2 + m2)))
    S = 0.0
    for v in range(vmin, vmax + 1):
        S += (-1) ** (v + j2 + m2) * f(j2 + j3 + m1 - v) * f(j1 - m1 + v) / (
            f(v) * f(j3 - j1 + j2 - v) * f(j3 + m3 - v) * f(v + j1 - j2 - m3))
    return C * S


def _q_real_to_complex(l):
    q = np.zeros((2 * l + 1, 2 * l + 1), dtype=np.complex128)
    for m in range(-l, 0):
        q[l + m, l + abs(m)] = 1 / 2 ** 0.5
        q[l + m, l - abs(m)] = -1j / 2 ** 0.5
    q[l, l] = 1.0
    for m in range(1, l + 1):
        q[l + m, l + abs(m)] = (-1) ** m / 2 ** 0.5
        q[l + m, l - abs(m)] = 1j * (-1) ** m / 2 ** 0.5
    return (-1j) ** l * q


def _wigner_3j(l1, l2, l3):
    C = np.zeros((2 * l1 + 1, 2 * l2 + 1, 2 * l3 + 1))
    for m1 in range(-l1, l1 + 1):
        for m2 in range(-l2, l2 + 1):
            m3 = m1 + m2
            if abs(m3) <= l3:
                C[l1 + m1, l2 + m2, l3 + m3] = _su2_cg(l1, m1, l2, m2, l3, m3)
    Q1, Q2, Q3 = _q_real_to_complex(l1), _q_real_to_complex(l2), _q_real_to_complex(l3)
    Cr = np.einsum('ij,kl,mn,ikm->jln', Q1, Q2, np.conj(Q3), C.astype(np.complex128))
    assert np.abs(Cr.imag).max() < 1e-6
    Cr = Cr.real
    return Cr / np.linalg.norm(Cr)


W3J = {p: _wigner_3j(*p) for p in set(PATHS)}
FAN = {lo: sum(1 for p in PATHS if p[2] == lo) * MUL * MUL for lo in (0, 1, 2)}
ALPHA = {lo: math.sqrt(2 * lo + 1) / math.sqrt(FAN[lo]) for lo in (0, 1, 2)}
QCART = np.concatenate([math.sqrt(2 * l + 1) * np.transpose(_wigner_3j(1, 1, l), (2, 0, 1))
                        for l in (0, 1, 2)], axis=0)  # (9, 3, 3)

KOFF = {0: 0, 1: 1, 2: 4}
FOFF = {0: 0, 1: MUL, 2: 4 * MUL}
IDIM = {0: 1, 1: 3, 2: 5}


def _build_B(tp_weights):
    """B[k, fa, fb]: irr[e,k] = x_a . B_k . x_b  (k = 9 irrep components)."""
    B = np.zeros((9, FDIM, FDIM), np.float64)
    for p, (l1, l2, lo) in enumerate(PATHS):
        w3j = W3J[(l1, l2, lo)]                      # (2l1+1, 2l2+1, 2lo+1)
        Wp = tp_weights[p].astype(np.float64)        # (u, v)
        contrib = np.einsum('uv,ijk->kuivj', Wp, w3j) * ALPHA[lo]
        B[KOFF[lo]:KOFF[lo] + IDIM[lo],
          FOFF[l1]:FOFF[l1] + MUL * IDIM[l1],
          FOFF[l2]:FOFF[l2] + MUL * IDIM[l2]] += contrib.reshape(
              IDIM[lo], MUL * IDIM[l1], MUL * IDIM[l2])
    return B


def _build_CT(tp_weights):
    """CT[(i*3+j), fb, fa] = C_ij^T with symmetrization folded in."""
    B = _build_B(tp_weights)
    Cfull = np.einsum('kab,kij->ijab', B, QCART)          # (3,3,fa,fb)
    Csym = 0.5 * (Cfull + np.transpose(Cfull, (1, 0, 3, 2)))
    CT = np.transpose(Csym, (0, 1, 3, 2)).reshape(9, FDIM, FDIM)
    return np.ascontiguousarray(CT.astype(np.float32))


def _canonical_layout():
    ii, jj = np.meshgrid(np.arange(N_ATOM, dtype=np.int32),
                         np.arange(N_ATOM, dtype=np.int32), indexing='ij')
    base = (np.arange(B_CFG, dtype=np.int32) * N_ATOM)[:, None, None]
    rows = (base + ii[None]).reshape(-1)
    cols = (base + jj[None]).reshape(-1)
    return np.stack([rows, cols], axis=1)


def _fallback_numpy(node_feats, layout, tp_weights):
    """General-layout path (host): exact reference semantics in numpy."""
    B = _build_B(tp_weights).astype(np.float32)
    x1 = node_feats[layout[:, 0]]
    x2 = node_feats[layout[:, 1]]
    irr = np.einsum('ef,kfg,eg->ek', x1, B, x2, optimize=True)
    cart = np.einsum('ek,kij->eij', irr, QCART.astype(np.float32))
    c = cart.reshape(B_CFG, N_ATOM, N_ATOM, 3, 3)
    c = 0.5 * (c + np.transpose(c, (0, 2, 1, 4, 3)))
    return np.ascontiguousarray(c.reshape(N_PAIRS, 3, 3))


# ---- device kernel ----
_NC = None
LAST_EXEC_NS = None
LAST_RES = None
PROFILE = False


def _install_profile_hook():
    """Self-install the axon NTFF profile hook (test-harness only).

    The agent image's antenv lacks axon_hooks; provide an in-process
    registry module and wire it to the injected libaxon_pjrt.so.
    """
    import types

    if "antenv.axon_hooks" not in sys.modules:
        import antenv
        mod = types.ModuleType("antenv.axon_hooks")
        state = {"h": None}
        mod.set_axon_ntff_profile_hook = lambda h: state.__setitem__("h", h)
        mod.get_axon_ntff_profile_hook = lambda: state["h"]
        sys.modules["antenv.axon_hooks"] = mod
        antenv.axon_hooks = mod
    import antenv.axon_hooks as ah
    if ah.get_axon_ntff_profile_hook() is None:
        from trn_agent_boot.trn_boot import _ntff_profile_via_ctypes
        ah.set_axon_ntff_profile_hook(
            _ntff_profile_via_ctypes("/opt/axon/libaxon_pjrt.so"))
    import concourse.bass_utils as bu
    bu.upload_artifacts = lambda d: d


def _build_nc():
    from concourse import bacc, mybir
    import concourse.tile as tile

    f32 = mybir.dt.float32
    f32r = mybir.dt.float32r

    nc = bacc.Bacc("TRN2", target_bir_lowering=False, debug=False,
                   num_devices=N_CORES)
    xt_ap = nc.dram_tensor("xt", [FDIM, ROWS_PER_CORE], f32r,
                           kind="ExternalInput").ap()
    ct_ap = nc.dram_tensor("ct", [9, FDIM, FDIM], f32r,
                           kind="ExternalInput").ap()
    out_ap = nc.dram_tensor("out", [ROWS_PER_CORE, N_ATOM * 9], f32,
                            kind="ExternalOutput").ap()

    C = CFG_PER_CORE   # 8 configs
    NB = N_ATOM        # 64
    NCOL = NB * 9      # 576 output cols per config

    with tile.TileContext(nc) as tc:
        with tc.tile_pool(name="const", bufs=1) as const_pool, \
             tc.tile_pool(name="u", bufs=1) as u_pool, \
             tc.tile_pool(name="outp", bufs=2) as outp, \
             tc.tile_pool(name="ps1", bufs=2, space="PSUM") as ps1, \
             tc.tile_pool(name="ps2", bufs=2, space="PSUM") as ps2:

            xt0 = const_pool.tile([128, ROWS_PER_CORE], f32r)
            xt1 = const_pool.tile([16, ROWS_PER_CORE], f32r)
            nc.sync.dma_start(xt0[:], xt_ap[0:128, :])
            nc.sync.dma_start(xt1[:], xt_ap[128:FDIM, :])

            ct0 = const_pool.tile([128, 9 * FDIM], f32r)
            ct1 = const_pool.tile([16, 9 * FDIM], f32r)
            for ij in range(9):
                nc.sync.dma_start(ct0[:, ij * FDIM:(ij + 1) * FDIM],
                                  ct_ap[ij, 0:128, :])
                nc.sync.dma_start(ct1[:, ij * FDIM:(ij + 1) * FDIM],
                                  ct_ap[ij, 128:FDIM, :])

            # stage 1: U_ij = C_ij @ X^T for all 8 configs at once
            # u layout: partitions = fa, free = (cfg, b, ij)
            u0 = u_pool.tile([128, C * NB * 9], f32r)
            u1 = u_pool.tile([16, C * NB * 9], f32r)
            u0v = u0[:].rearrange("p (c b j) -> p c b j", c=C, b=NB)
            u1v = u1[:].rearrange("p (c b j) -> p c b j", c=C, b=NB)
            for ij in range(9):
                o = ij * FDIM
                pa = ps1.tile([128, ROWS_PER_CORE], f32, tag="pa")
                nc.tensor.matmul(pa[:], ct0[:, o:o + 128], xt0[:],
                                 start=True, stop=False)
                nc.tensor.matmul(pa[:], ct1[:, o:o + 128], xt1[:],
                                 start=False, stop=True)
                pb = ps1.tile([16, ROWS_PER_CORE], f32, tag="pb")
                nc.tensor.matmul(pb[:], ct0[:, o + 128:o + FDIM], xt0[:],
                                 start=True, stop=False)
                nc.tensor.matmul(pb[:], ct1[:, o + 128:o + FDIM], xt1[:],
                                 start=False, stop=True)
                eng = nc.scalar if ij % 2 == 0 else nc.vector
                eng2 = nc.vector if ij % 2 == 0 else nc.scalar
                pav = pa[:].rearrange("p (c b) -> p c b", c=C)
                pbv = pb[:].rearrange("p (c b) -> p c b", c=C)
                if ij % 2 == 0:
                    eng.copy(u0v[:, :, :, ij], pav)
                    eng2.tensor_copy(u1v[:, :, :, ij], pbv)
                else:
                    eng.tensor_copy(u0v[:, :, :, ij], pav)
                    eng2.copy(u1v[:, :, :, ij], pbv)

            # stage 2: F = X_c @ U_c -> out rows (c,a), cols (b,ij)
            H = NCOL // 2  # 288
            for c in range(C):
                lhs0 = xt0[:, c * NB:(c + 1) * NB]
                lhs1 = xt1[:, c * NB:(c + 1) * NB]
                base = c * NCOL
                pf0 = ps2.tile([NB, H], f32, tag="pf0")
                nc.tensor.matmul(pf0[:], lhs0, u0[:, base:base + H],
                                 start=True, stop=False)
                nc.tensor.matmul(pf0[:], lhs1, u1[:, base:base + H],
                                 start=False, stop=True)
                pf1 = ps2.tile([NB, H], f32, tag="pf1")
                nc.tensor.matmul(pf1[:], lhs0, u0[:, base + H:base + NCOL],
                                 start=True, stop=False)
                nc.tensor.matmul(pf1[:], lhs1, u1[:, base + H:base + NCOL],
                                 start=False, stop=True)
                fo = outp.tile([NB, NCOL], f32)
                if c % 2 == 0:
                    nc.scalar.copy(fo[:, 0:H], pf0[:])
                    nc.vector.tensor_copy(fo[:, H:NCOL], pf1[:])
                else:
                    nc.vector.tensor_copy(fo[:, 0:H], pf0[:])
                    nc.scalar.copy(fo[:, H:NCOL], pf1[:])
                nc.sync.dma_start(out_ap[c * NB:(c + 1) * NB, :], fo[:])

    nc.compile()
    return nc


def kernel(node_feats, layout, tp_weights):
    global _NC, LAST_EXEC_NS
    node_feats = np.ascontiguousarray(np.asarray(node_feats, dtype=np.float32))
    layout = np.asarray(layout, dtype=np.int32)
    tp_weights = np.asarray(tp_weights, dtype=np.float32)

    if not np.array_equal(layout, _canonical_layout()):
        return _fallback_numpy(node_feats, layout, tp_weights)

    CT = _build_CT(tp_weights)

    if _NC is None:
        _NC = _build_nc()

    from concourse.bass_utils import run_bass_kernel_spmd

    if PROFILE:
        try:
            _install_profile_hook()
        except Exception:
            pass

    in_maps = []
    for m in range(N_CORES):
        sl = node_feats[m * ROWS_PER_CORE:(m + 1) * ROWS_PER_CORE, :]
        xt = np.ascontiguousarray(sl.T)          # (144, 512)
        in_maps.append({"xt": xt, "ct": CT})

    res = run_bass_kernel_spmd(_NC, in_maps, list(range(N_CORES)),
                               trace=PROFILE)
    LAST_EXEC_NS = res.exec_time_ns
    globals()['LAST_RES'] = res

    out = np.empty((N_PAIRS, 3, 3), dtype=np.float32)
    for m in range(N_CORES):
        blk = res.results[m]["out"]              # (512, 576)
        out[m * ROWS_PER_CORE * N_ATOM:(m + 1) * ROWS_PER_CORE * N_ATOM] = \
            blk.reshape(ROWS_PER_CORE * N_ATOM, 3, 3)
    return out
